# revision 14
# baseline (speedup 1.0000x reference)
"""Swin-style 3D windowed attention (B=32, N=513, C=768, H=12) on 8 TRN2 cores.

Strategy: pure data-parallel over batch (4 batches/core, no collectives).
Host side does input marshalling only: bf16 casts, x transpose, the static
relative-position bias gather exp(table[idx]) (index metadata), and the 1/8
q-scale folded into w_qkv.  Device does qkv projection, S^T = K^T.T@Q^T
attention scores, exp (no max-sub needed: scores bounded ~|2.5|), bias via
multiplicative exp(bias), P@V with a ones-column to get softmax sums free,
normalization, and output projection, all in bf16 matmuls / fp32 softmax.
"""

import numpy as np
import ml_dtypes

import concourse.bass as bass
import concourse.mybir as mybir
import concourse.tile as tile
from concourse import bacc
from concourse.bass_utils import run_bass_kernel_spmd
from concourse.masks import make_identity

B, N, C, H, Dh = 32, 513, 768, 12, 64
NCORES = 8
BC = B // NCORES           # 4 batches per core
M = BC * N                 # 2052 rows per core
KC = C // 128              # 6 contraction chunks
QKVC = 3 * C // 128        # 18 qkv feature chunks
BF16 = mybir.dt.bfloat16
F32 = mybir.dt.float32

_nc_cache = {}


def _ceil_chunks(total, step):
    out = []
    o = 0
    while o < total:
        out.append((o, min(step, total - o)))
        o += step
    return out


M_CHUNKS = _ceil_chunks(M, 512)       # [(0,512)x4, (2048,4)]
J_CHUNKS = _ceil_chunks(N, 128)       # [(0,128)x4, (512,1)]
I_CHUNKS_MM = _ceil_chunks(N, 512)    # [(0,512), (512,1)]
I_CHUNKS = _ceil_chunks(N, 128)
PROJ_N_CHUNKS = _ceil_chunks(C, 512)  # [(0,512), (512,256)]
PROJ_M_CHUNKS = _ceil_chunks(M, 128)


def build_bass():
    nc = bacc.Bacc(None, target_bir_lowering=False, debug=False)

    xT = nc.declare_dram_parameter("xT", [C, M], BF16, isOutput=False)
    w = nc.declare_dram_parameter("w", [C, 3 * C], BF16, isOutput=False)
    wp = nc.declare_dram_parameter("wp", [C, C], BF16, isOutput=False)
    bp = nc.declare_dram_parameter("bp", [1, C], F32, isOutput=False)
    eb = nc.declare_dram_parameter("eb", [H, N, N], BF16, isOutput=False)
    idv = nc.declare_dram_parameter("idv", [128, Dh], BF16, isOutput=False)
    out = nc.declare_dram_parameter("out", [M, C], F32, isOutput=True)

    with tile.TileContext(nc) as tc:
        with (
            tc.tile_pool(name="persist", bufs=1) as pp,
            tc.tile_pool(name="work", bufs=3) as wk,
            tc.tile_pool(name="psum", bufs=2, space="PSUM") as ps,
        ):
            # ---- persistent sbuf tensors ----
            w_sb = pp.tile([128, KC, 3 * C], BF16)        # 3.5 MB
            wp_sb = pp.tile([128, KC, C], BF16)           # 1.2 MB
            bp_sb = pp.tile([128, C], F32)                # 0.4 MB
            qkvT = pp.tile([128, QKVC, M], BF16)          # 9.4 MB
            aoT = pp.tile([128, KC, M], BF16)             # 3.2 MB
            ident = pp.tile([128, 128], BF16)
            idv_sb = pp.tile([128, Dh], BF16)

            make_identity(nc, ident[:, :])
            nc.sync.dma_start(out=idv_sb[:, :], in_=idv[:, :])

            nc.sync.dma_start(
                out=w_sb[:, :, :],
                in_=w.rearrange("(a p) n -> p a n", p=128),
            )
            nc.sync.dma_start(
                out=wp_sb[:, :, :],
                in_=wp.rearrange("(a p) n -> p a n", p=128),
            )
            nc.sync.dma_start(
                out=bp_sb[:, :],
                in_=bass.AP(tensor=bp, offset=0, ap=[[0, 128], [1, C]]),
            )

            # ---- phase 1: qkvT[c, m] = sum_k w[k, c] * xT[k, m] ----
            for mo, mw in M_CHUNKS:
                xt_tiles = []
                for kk in range(KC):
                    xt = wk.tile([128, 512], BF16, tag="xt", bufs=6)
                    nc.gpsimd.dma_start(
                        out=xt[:, :mw], in_=xT[kk * 128:(kk + 1) * 128, mo:mo + mw]
                    )
                    xt_tiles.append(xt)
                for cc in range(QKVC):
                    pt = ps.tile([128, 512], F32, tag="mm", bufs=2)
                    for kk in range(KC):
                        nc.tensor.matmul(
                            pt[:, :mw],
                            w_sb[:, kk, cc * 128:(cc + 1) * 128],
                            xt_tiles[kk][:, :mw],
                            start=(kk == 0),
                            stop=(kk == KC - 1),
                        )
                    nc.vector.tensor_copy(qkvT[:, cc, mo:mo + mw], pt[:, :mw])

            # ---- phase 2: attention, loop h outer (bias reuse), b inner ----
            for h in range(H):
                r0 = 64 * (h % 2)
                qc, kc_, vc = h // 2, 6 + h // 2, 12 + h // 2
                eb_t = wk.tile([128, len(J_CHUNKS), N], BF16, tag="eb", bufs=2)
                for jc, (jo, jw) in enumerate(J_CHUNKS):
                    nc.sync.dma_start(
                        out=eb_t[:jw, jc, :], in_=eb[h, jo:jo + jw, :]
                    )
                for b in range(BC):
                    col0 = b * N
                    # V natural [j, d] (+ ones column 64) via PE transpose
                    v_sb = wk.tile([128, len(J_CHUNKS), Dh + 1], BF16,
                                   tag="v", bufs=2)
                    nc.vector.memset(v_sb[:, :, :], 1.0)
                    for jc, (jo, jw) in enumerate(J_CHUNKS):
                        vt_ps = ps.tile([128, Dh], BF16, tag="vtp", bufs=2)
                        nc.tensor.transpose(
                            vt_ps[:jw, :],
                            qkvT[r0:r0 + 64, vc, col0 + jo:col0 + jo + jw],
                            idv_sb[r0:r0 + 64, :],
                        )
                        nc.vector.tensor_copy(
                            v_sb[:jw, jc, 0:Dh], vt_ps[:jw, :]
                        )
                    # S^T tiles + exp + bias-mul
                    et = wk.tile([128, len(J_CHUNKS), N], BF16, tag="et", bufs=2)
                    for jc, (jo, jw) in enumerate(J_CHUNKS):
                        st = ps.tile([128, N], F32, tag="st", bufs=1)
                        for io, iw in I_CHUNKS_MM:
                            nc.tensor.matmul(
                                st[:jw, io:io + iw],
                                qkvT[r0:r0 + 64, kc_, col0 + jo:col0 + jo + jw],
                                qkvT[r0:r0 + 64, qc, col0 + io:col0 + io + iw],
                                start=True,
                                stop=True,
                            )
                        nc.scalar.activation(
                            out=et[:jw, jc, :],
                            in_=st[:jw, :],
                            func=mybir.ActivationFunctionType.Exp,
                        )
                        nc.vector.tensor_mul(
                            et[:jw, jc, :], et[:jw, jc, :], eb_t[:jw, jc, :]
                        )
                    # out_unnorm[i, d(+sum)] = sum_j E^T[j,i] * v'[j,d]
                    for ic, (io, iw) in enumerate(I_CHUNKS):
                        pv = ps.tile([128, Dh + 1], F32, tag="pv", bufs=2)
                        for jc, (jo, jw) in enumerate(J_CHUNKS):
                            nc.tensor.matmul(
                                pv[:iw, :],
                                et[:jw, jc, io:io + iw],
                                v_sb[:jw, jc, :],
                                start=(jc == 0),
                                stop=(jc == len(J_CHUNKS) - 1),
                            )
                        rc = wk.tile([128, 1], F32, tag="rc", bufs=2)
                        nc.vector.reciprocal(rc[:iw, :], pv[:iw, Dh:Dh + 1])
                        ao = wk.tile([128, Dh], BF16, tag="ao", bufs=2)
                        nc.vector.tensor_scalar_mul(
                            ao[:iw, :], pv[:iw, 0:Dh], rc[:iw, :]
                        )
                        aot_ps = ps.tile([128, 128], BF16, tag="vtp", bufs=2)
                        nc.tensor.transpose(
                            aot_ps[0:Dh, :iw], ao[:iw, :], ident[:iw, :iw]
                        )
                        nc.vector.tensor_copy(
                            aoT[r0:r0 + 64, h // 2, col0 + io:col0 + io + iw],
                            aot_ps[0:Dh, :iw],
                        )

            # ---- phase 3: out = aoT.T @ wp + bp ----
            for mo, mw in PROJ_M_CHUNKS:
                for no, nw in PROJ_N_CHUNKS:
                    pt = ps.tile([128, 512], F32, tag="mm", bufs=2)
                    for kk in range(KC):
                        nc.tensor.matmul(
                            pt[:mw, :nw],
                            aoT[:, kk, mo:mo + mw],
                            wp_sb[:, kk, no:no + nw],
                            start=(kk == 0),
                            stop=(kk == KC - 1),
                        )
                    ot = wk.tile([128, 512], F32, tag="ot", bufs=3)
                    nc.vector.tensor_add(
                        ot[:mw, :nw], pt[:mw, :nw], bp_sb[:mw, no:no + nw]
                    )
                    nc.sync.dma_start(
                        out=out[mo:mo + mw, no:no + nw], in_=ot[:mw, :nw]
                    )
    nc.compile()
    return nc


def _prep_inputs(x, w_qkv, w_proj, b_proj, rel_bias_table, rel_pos_index):
    bf = ml_dtypes.bfloat16
    w_host = np.asarray(w_qkv, np.float32).copy()
    w_host[:, :C] *= 0.125  # fold q scale (exact power of two)
    w_host = w_host.astype(bf)
    wp_host = np.asarray(w_proj, np.float32).astype(bf)
    bp_host = np.asarray(b_proj, np.float32).reshape(1, C)
    g = np.asarray(rel_bias_table, np.float32)[np.asarray(rel_pos_index)]
    eb_host = np.exp(g).transpose(2, 0, 1).copy().astype(bf)  # [H, j, i]
    xs = np.asarray(x, np.float32).reshape(NCORES, M, C)
    in_maps = []
    for c in range(NCORES):
        xT_c = np.ascontiguousarray(xs[c].astype(bf).T)
        idv_host = np.tile(np.eye(Dh, dtype=np.float32), (2, 1)).astype(bf)
        in_maps.append({
            "xT": xT_c, "w": w_host, "wp": wp_host, "bp": bp_host,
            "eb": eb_host, "idv": idv_host,
        })
    return in_maps


def run(inputs, trace=False):
    if "nc" not in _nc_cache:
        _nc_cache["nc"] = build_bass()
    nc = _nc_cache["nc"]
    in_maps = _prep_inputs(**inputs)
    res = run_bass_kernel_spmd(
        nc, in_maps, core_ids=list(range(NCORES)), trace=trace
    )
    outs = [np.asarray(r["out"], np.float32).reshape(BC, N, C)
            for r in res.results]
    return np.concatenate(outs, axis=0), res


def kernel(**inputs) -> np.ndarray:
    full, _ = run(inputs, trace=False)
    return full


# revision 15
# speedup vs baseline: 1.0188x; 1.0188x over previous
"""Swin-style 3D windowed attention (B=32, N=513, C=768, H=12) on 8 TRN2 cores.

Strategy: pure data-parallel over batch (4 batches/core, no collectives).
Host side does input marshalling only: bf16 casts, x transpose, the static
relative-position bias gather exp(table[idx]) (index metadata), and the 1/8
q-scale folded into w_qkv.  Device does qkv projection, S^T = K^T.T@Q^T
attention scores, exp (no max-sub needed: scores bounded ~|2.5|), bias via
multiplicative exp(bias), P@V with a ones-column to get softmax sums free,
normalization, and output projection, all in bf16 matmuls / fp32 softmax.
"""

import numpy as np
import ml_dtypes

import concourse.bass as bass
import concourse.mybir as mybir
import concourse.tile as tile
from concourse import bacc
from concourse.bass_utils import run_bass_kernel_spmd
from concourse.masks import make_identity

B, N, C, H, Dh = 32, 513, 768, 12, 64
NCORES = 8
BC = B // NCORES           # 4 batches per core
M = BC * N                 # 2052 rows per core
KC = C // 128              # 6 contraction chunks
QKVC = 3 * C // 128        # 18 qkv feature chunks
BF16 = mybir.dt.bfloat16
F32 = mybir.dt.float32

_nc_cache = {}


def _ceil_chunks(total, step):
    out = []
    o = 0
    while o < total:
        out.append((o, min(step, total - o)))
        o += step
    return out


M_CHUNKS = _ceil_chunks(M, 512)       # [(0,512)x4, (2048,4)]
J_CHUNKS = _ceil_chunks(N, 128)       # [(0,128)x4, (512,1)]
I_CHUNKS_MM = _ceil_chunks(N, 512)    # [(0,512), (512,1)]
I_CHUNKS = _ceil_chunks(N, 128)
PROJ_N_CHUNKS = _ceil_chunks(C, 512)  # [(0,512), (512,256)]
PROJ_M_CHUNKS = _ceil_chunks(M, 128)


def build_bass():
    nc = bacc.Bacc(None, target_bir_lowering=False, debug=False)

    xT = nc.declare_dram_parameter("xT", [C, M], BF16, isOutput=False)
    w = nc.declare_dram_parameter("w", [C, 3 * C], BF16, isOutput=False)
    wp = nc.declare_dram_parameter("wp", [C, C], BF16, isOutput=False)
    bp = nc.declare_dram_parameter("bp", [1, C], F32, isOutput=False)
    eb = nc.declare_dram_parameter("eb", [H, N, N], BF16, isOutput=False)
    idv = nc.declare_dram_parameter("idv", [128, Dh], BF16, isOutput=False)
    out = nc.declare_dram_parameter("out", [M, C], F32, isOutput=True)

    with tile.TileContext(nc) as tc:
        with (
            tc.tile_pool(name="persist", bufs=1) as pp,
            tc.tile_pool(name="work", bufs=3) as wk,
            tc.tile_pool(name="psum", bufs=2, space="PSUM") as ps,
        ):
            # ---- persistent sbuf tensors ----
            w_sb = pp.tile([128, KC, 3 * C], BF16)        # 3.5 MB
            wp_sb = pp.tile([128, KC, C], BF16)           # 1.2 MB
            bp_sb = pp.tile([128, C], F32)                # 0.4 MB
            qkvT = pp.tile([128, QKVC, M], BF16)          # 9.4 MB
            aoT = pp.tile([128, KC, M], BF16)             # 3.2 MB
            ident = pp.tile([128, 128], BF16)
            idv_sb = pp.tile([128, Dh], BF16)

            make_identity(nc, ident[:, :])
            nc.sync.dma_start(out=idv_sb[:, :], in_=idv[:, :])

            nc.sync.dma_start(
                out=w_sb[:, :, :],
                in_=w.rearrange("(a p) n -> p a n", p=128),
            )
            nc.sync.dma_start(
                out=wp_sb[:, :, :],
                in_=wp.rearrange("(a p) n -> p a n", p=128),
            )
            nc.sync.dma_start(
                out=bp_sb[:, :],
                in_=bass.AP(tensor=bp, offset=0, ap=[[0, 128], [1, C]]),
            )

            # ---- phase 1: qkvT[c, m] = sum_k w[k, c] * xT[k, m] ----
            for mo, mw in M_CHUNKS:
                xt_tiles = []
                for kk in range(KC):
                    xt = wk.tile([128, 512], BF16, tag="xt", bufs=12)
                    nc.gpsimd.dma_start(
                        out=xt[:, :mw], in_=xT[kk * 128:(kk + 1) * 128, mo:mo + mw]
                    )
                    xt_tiles.append(xt)
                for cc in range(QKVC):
                    pt = ps.tile([128, 512], F32, tag="mm", bufs=2)
                    for kk in range(KC):
                        nc.tensor.matmul(
                            pt[:, :mw],
                            w_sb[:, kk, cc * 128:(cc + 1) * 128],
                            xt_tiles[kk][:, :mw],
                            start=(kk == 0),
                            stop=(kk == KC - 1),
                        )
                    nc.vector.tensor_copy(qkvT[:, cc, mo:mo + mw], pt[:, :mw])

            # ---- phase 2: attention, loop h outer (bias reuse), b inner ----
            for h in range(H):
                r0 = 64 * (h % 2)
                qc, kc_, vc = h // 2, 6 + h // 2, 12 + h // 2
                eb_t = wk.tile([128, len(J_CHUNKS), N], BF16, tag="eb", bufs=1)
                for jc, (jo, jw) in enumerate(J_CHUNKS):
                    nc.sync.dma_start(
                        out=eb_t[:jw, jc, :], in_=eb[h, jo:jo + jw, :]
                    )
                for b in range(BC):
                    col0 = b * N
                    # V natural [j, d] (+ ones column 64) via PE transpose
                    v_sb = wk.tile([128, len(J_CHUNKS), Dh + 1], BF16,
                                   tag="v", bufs=2)
                    nc.vector.memset(v_sb[:, :, :], 1.0)
                    for jc, (jo, jw) in enumerate(J_CHUNKS):
                        vt_ps = ps.tile([128, Dh], BF16, tag="vtp", bufs=2)
                        nc.tensor.transpose(
                            vt_ps[:jw, :],
                            qkvT[r0:r0 + 64, vc, col0 + jo:col0 + jo + jw],
                            idv_sb[r0:r0 + 64, :],
                        )
                        nc.vector.tensor_copy(
                            v_sb[:jw, jc, 0:Dh], vt_ps[:jw, :]
                        )
                    # S^T tiles + exp + bias-mul
                    et = wk.tile([128, len(J_CHUNKS), N], BF16, tag="et", bufs=3)
                    for jc, (jo, jw) in enumerate(J_CHUNKS):
                        st = ps.tile([128, N], F32, tag="st", bufs=1)
                        for io, iw in I_CHUNKS_MM:
                            nc.tensor.matmul(
                                st[:jw, io:io + iw],
                                qkvT[r0:r0 + 64, kc_, col0 + jo:col0 + jo + jw],
                                qkvT[r0:r0 + 64, qc, col0 + io:col0 + io + iw],
                                start=True,
                                stop=True,
                            )
                        nc.scalar.activation(
                            out=et[:jw, jc, :],
                            in_=st[:jw, :],
                            func=mybir.ActivationFunctionType.Exp,
                        )
                        nc.vector.tensor_mul(
                            et[:jw, jc, :], et[:jw, jc, :], eb_t[:jw, jc, :]
                        )
                    # out_unnorm[i, d(+sum)] = sum_j E^T[j,i] * v'[j,d]
                    for ic, (io, iw) in enumerate(I_CHUNKS):
                        pv = ps.tile([128, Dh + 1], F32, tag="pv", bufs=2)
                        for jc, (jo, jw) in enumerate(J_CHUNKS):
                            nc.tensor.matmul(
                                pv[:iw, :],
                                et[:jw, jc, io:io + iw],
                                v_sb[:jw, jc, :],
                                start=(jc == 0),
                                stop=(jc == len(J_CHUNKS) - 1),
                            )
                        rc = wk.tile([128, 1], F32, tag="rc", bufs=2)
                        nc.vector.reciprocal(rc[:iw, :], pv[:iw, Dh:Dh + 1])
                        ao = wk.tile([128, Dh], BF16, tag="ao", bufs=2)
                        nc.vector.tensor_scalar_mul(
                            ao[:iw, :], pv[:iw, 0:Dh], rc[:iw, :]
                        )
                        aot_ps = ps.tile([128, 128], BF16, tag="vtp", bufs=2)
                        nc.tensor.transpose(
                            aot_ps[0:Dh, :iw], ao[:iw, :], ident[:iw, :iw]
                        )
                        nc.vector.tensor_copy(
                            aoT[r0:r0 + 64, h // 2, col0 + io:col0 + io + iw],
                            aot_ps[0:Dh, :iw],
                        )

            # ---- phase 3: out = aoT.T @ wp + bp ----
            for mo, mw in PROJ_M_CHUNKS:
                for no, nw in PROJ_N_CHUNKS:
                    pt = ps.tile([128, 512], F32, tag="mm", bufs=2)
                    for kk in range(KC):
                        nc.tensor.matmul(
                            pt[:mw, :nw],
                            aoT[:, kk, mo:mo + mw],
                            wp_sb[:, kk, no:no + nw],
                            start=(kk == 0),
                            stop=(kk == KC - 1),
                        )
                    ot = wk.tile([128, 512], F32, tag="ot", bufs=3)
                    nc.vector.tensor_add(
                        ot[:mw, :nw], pt[:mw, :nw], bp_sb[:mw, no:no + nw]
                    )
                    nc.sync.dma_start(
                        out=out[mo:mo + mw, no:no + nw], in_=ot[:mw, :nw]
                    )
    nc.compile()
    return nc


def _prep_inputs(x, w_qkv, w_proj, b_proj, rel_bias_table, rel_pos_index):
    bf = ml_dtypes.bfloat16
    w_host = np.asarray(w_qkv, np.float32).copy()
    w_host[:, :C] *= 0.125  # fold q scale (exact power of two)
    w_host = w_host.astype(bf)
    wp_host = np.asarray(w_proj, np.float32).astype(bf)
    bp_host = np.asarray(b_proj, np.float32).reshape(1, C)
    g = np.asarray(rel_bias_table, np.float32)[np.asarray(rel_pos_index)]
    eb_host = np.exp(g).transpose(2, 0, 1).copy().astype(bf)  # [H, j, i]
    xs = np.asarray(x, np.float32).reshape(NCORES, M, C)
    in_maps = []
    for c in range(NCORES):
        xT_c = np.ascontiguousarray(xs[c].astype(bf).T)
        idv_host = np.tile(np.eye(Dh, dtype=np.float32), (2, 1)).astype(bf)
        in_maps.append({
            "xT": xT_c, "w": w_host, "wp": wp_host, "bp": bp_host,
            "eb": eb_host, "idv": idv_host,
        })
    return in_maps


def run(inputs, trace=False):
    if "nc" not in _nc_cache:
        _nc_cache["nc"] = build_bass()
    nc = _nc_cache["nc"]
    in_maps = _prep_inputs(**inputs)
    res = run_bass_kernel_spmd(
        nc, in_maps, core_ids=list(range(NCORES)), trace=trace
    )
    outs = [np.asarray(r["out"], np.float32).reshape(BC, N, C)
            for r in res.results]
    return np.concatenate(outs, axis=0), res


def kernel(**inputs) -> np.ndarray:
    full, _ = run(inputs, trace=False)
    return full
